# revision 4
# baseline (speedup 1.0000x reference)
"""Trainium2 Bass kernel for nn_DentalAnatomyLoss.

Computes, for segmentation [B=2, C=32, D=64, H=128, W=128] fp32:
  - crown/root ratio loss (per (b,c) sums over d<32 / d>=32)
  - 3D total-variation loss (mean |diff| along w, h, d)
  - returns stack([crown_root, smoothness, total_anatomy]) fp32 [3]

Strategy: pure data-parallel over the 64 (b,c) slices, 8 per NeuronCore.
Each core reduces its 32 MiB shard to a [128, 52] fp32 partial tensor;
the host combines partials into the 3 scalars.

Layout: d-on-partitions. Each "chunk pair" (cp) holds 2 slices:
partition p = s*64 + d for local slice s in {0,1}, plane d in 0..63;
free axis = (h, w) = 16384 bf16. Benefits over the h-partition layout:
  - DMA reads are 16 KiB contiguous per partition (vs 512 B rows), and
    the fp32->bf16 cast happens inside the SWDGE DMA (measured at full
    HBM rate), freeing ScalarE entirely from casting.
  - The h-diff (gy) becomes an aligned free-axis shift by w -> one fused
    scalar_tensor_tensor max+accum per cp on VectorE.
  - The d-diff (gz) is the partition-axis diff -> TensorE block-bidiag
    matmul into PSUM (columns 63/127 zeroed so no cross-slice pairs),
    drained by ScalarE Abs+accum. Rows 63/127 drain |0| = 0.

Per-core engine budget (measured sustained rates):
  VectorE ~136 us: gx + gy fused STT max+accum (1x; sweeping 2x modes
    does not help: any elementwise+reduce pair costs the same 2 touches).
  ScalarE ~131 us: per-plane sum(x) via broadcast-out Copy+accum (fp32
    exact, feeds crown/root and the max-trick telescopes), PSUM drains,
    and the tiny first/last row/col telescope sums.
  TensorE ~70 us, DMA ~100 us (HBM roofline ~94 us/core).

Host recovers sum|a-b| = 2*sum(max(a,b)) - sum(a) - sum(b); the signed
sums telescope to per-plane sums and first/last row/col sums. gx and gy
share one denominator (d*h*(w-1) == d*(h-1)*w), gz has its own.
"""

import os

import numpy as np

B, C, D, H, W = 2, 32, 64, 128, 128
NCORES = 8
JPC = (B * C) // NCORES  # slices per core
CROWN_ROOT_W = 2.0
SMOOTH_W = 1.5
EXPECTED_RATIO = 1.2

# accumulator column layout in the [128, ACC_COLS] partial tensor
NCP = JPC // 2  # chunk pairs per core
COL_SX = 0  # NCP: per-plane sum(x)
COL_GY = COL_SX + NCP  # NCP: per-plane sum(max(x[h+1], x[h]))
COL_GX = COL_GY + NCP  # NCP: per-plane sum(max(x[w+1], x[w]))
COL_R = COL_GX + NCP  # NCP: per-plane sum(row0 + row127)
COL_C = COL_R + NCP  # NCP: per-plane sum(col0 + col127)
COL_DZ = COL_C + NCP  # NCP*NDRAIN: PSUM |dz| drains
NDRAIN = 8
ACC_COLS = COL_DZ + NCP * NDRAIN

_PROG_CACHE: dict = {}
last_exec_time_ns = None


def _build_program(jpc=JPC, d=D, h=H, w=W, repeat=1, skip=()):
    """Build the (single) SPMD Bass program run identically on all cores.

    repeat>1 wraps the whole compute in a hardware For_i loop (identical
    result, used only for wall-clock timing of the kernel body).
    """
    from contextlib import ExitStack

    import concourse.tile as tile
    from concourse import bacc, mybir

    f32 = mybir.dt.float32
    bf16 = mybir.dt.bfloat16
    AO = mybir.AluOpType
    AF = mybir.ActivationFunctionType

    ncp = jpc // 2
    P = 2 * d  # partitions per chunk pair
    fsz = h * w  # free size per partition (one (h,w) plane)
    nq = 4  # DMA splits per chunk pair
    qsz = fsz // nq
    nblk = fsz // 512  # 512-col matmul blocks per cp
    ndrain = NDRAIN if fsz == 16384 else max(1, nblk // 4)
    blk_per_drain = nblk // ndrain
    dsz = blk_per_drain * 512  # free size of one PSUM drain tile

    acc_cols = COL_C + ncp + ncp * ndrain
    col_dz = COL_C + ncp

    nc = bacc.Bacc(
        "TRN2",
        target_bir_lowering=False,
        debug=False,
        enable_asserts=False,
        num_devices=NCORES,
    )
    seg = nc.dram_tensor("seg", [jpc, d, h, w], f32, kind="ExternalInput").ap()
    bd = nc.dram_tensor("bidiag", [P, P], f32, kind="ExternalInput").ap()
    out = nc.dram_tensor("partials", [P, acc_cols], f32, kind="ExternalOutput").ap()

    with tile.TileContext(nc) as tc, ExitStack() as ctx:
        singles = ctx.enter_context(tc.tile_pool(name="singles", bufs=1))
        xbp = ctx.enter_context(tc.tile_pool(name="xb", bufs=2))
        scrp = ctx.enter_context(tc.tile_pool(name="scr", bufs=2))
        dumbp = ctx.enter_context(tc.tile_pool(name="dumb", bufs=2))
        psp = ctx.enter_context(tc.tile_pool(name="ps", bufs=2, space="PSUM"))

        bd_sb = singles.tile([P, P], f32)
        nc.sync.dma_start(out=bd_sb, in_=bd)
        acc = singles.tile([P, acc_cols], f32)
        nc.vector.memset(acc, 0.0)

        def cp_body(c):
            # 1) HWDGE fp32 loads, d-layout. Per partition: contiguous
            #    4*qsz bytes from DRAM. (HWDGE transfers overlap across
            #    queues; the SWDGE cast path serializes and is ~3x
            #    slower end-to-end, so everything stays fp32 on-chip.)
            xb = xbp.tile([P, fsz], f32)
            src = seg[2 * c : 2 * c + 2].rearrange("s d h w -> (s d) (h w)")
            for q in range(nq):
                nc.sync.dma_start(
                    out=xb[:, q * qsz : (q + 1) * qsz],
                    in_=src[:, q * qsz : (q + 1) * qsz],
                )

            scratch = scrp.tile([P, fsz - w], bf16)
            dummy = dumbp.tile([P, 1], bf16)

            # 2) VectorE: fused max+accum for gy (aligned shift by w) and
            #    gx (shift by 1 inside each w-row). Both 1x; one op each.
            if "gy" not in skip:
                nc.vector.scalar_tensor_tensor(
                    out=scratch,
                    in0=xb[:, w:fsz],
                    scalar=0.0,
                    in1=xb[:, 0 : fsz - w],
                    op0=AO.bypass,
                    op1=AO.max,
                    accum_out=acc[:, COL_GY + c : COL_GY + c + 1],
                )
            if "gx" not in skip:
                xb3 = xb.rearrange("p (r c2) -> p r c2", c2=w)
                scr3 = scratch.rearrange("p (r c2) -> p r c2", c2=w - 1)[
                    :, 0:h, :
                ]
                nc.vector.scalar_tensor_tensor(
                    out=scr3,
                    in0=xb3[:, :, 1:w],
                    scalar=0.0,
                    in1=xb3[:, :, 0 : w - 1],
                    op0=AO.bypass,
                    op1=AO.max,
                    accum_out=acc[:, COL_GX + c : COL_GX + c + 1],
                )

            # 3) ScalarE: per-plane sum(x) (exact fp32 accum); telescope
            #    row/col sums; all via broadcast-out Copy+accum.
            if "sx" not in skip:
                nc.scalar.activation(
                    out=dummy.broadcast_to((P, fsz)),
                    in_=xb,
                    func=AF.Copy,
                    accum_out=acc[:, COL_SX + c : COL_SX + c + 1],
                )
                # sum(row0 + row_{h-1}) per plane
                rows = xb.rearrange("p (r c2) -> p r c2", c2=w)[
                    :, 0 : h : h - 1, :
                ]
                nc.scalar.activation(
                    out=dummy.broadcast_to((P, 2, w)),
                    in_=rows,
                    func=AF.Copy,
                    accum_out=acc[:, COL_R + c : COL_R + c + 1],
                )
                # sum(col0 + col_{w-1}) per plane
                cols = xb.rearrange("p (r c2) -> p c2 r", c2=w)[
                    :, 0 : w : w - 1, :
                ]
                nc.scalar.activation(
                    out=dummy.broadcast_to((P, 2, h)),
                    in_=cols,
                    func=AF.Copy,
                    accum_out=acc[:, COL_C + c : COL_C + c + 1],
                )

            # 4) TensorE block-bidiag d-diffs -> PSUM; ScalarE Abs drains.
            if "gz" not in skip:
                for t in range(ndrain):
                    ps = psp.tile([P, blk_per_drain, 512], f32)
                    for b in range(blk_per_drain):
                        blk = t * blk_per_drain + b
                        nc.tensor.matmul(
                            ps[:, b, :],
                            bd_sb,
                            xb[:, blk * 512 : (blk + 1) * 512],
                            start=True,
                            stop=True,
                        )
                    col = col_dz + ndrain * c + t
                    nc.scalar.activation(
                        out=dummy.broadcast_to((P, blk_per_drain, 512)),
                        in_=ps[:, :, :],
                        func=AF.Abs,
                        accum_out=acc[:, col : col + 1],
                    )

        def all_cps():
            for c in range(ncp):
                cp_body(c)

        if repeat == 1:
            all_cps()
        else:
            with tc.For_i(0, repeat, 1):
                all_cps()
        nc.sync.dma_start(out=out, in_=acc)

    nc.compile()
    return nc


def _get_program():
    key = "full"
    if key not in _PROG_CACHE:
        _PROG_CACHE[key] = _build_program()
    return _PROG_CACHE[key]


def _bidiag_np(d=D):
    """lhsT for the d-diff matmul: out[m,:] = x[m+1,:] - x[m,:] within
    each slice; columns d-1 and 2d-1 zeroed (no cross-slice pairs)."""
    P = 2 * d
    m = np.zeros((P, P), dtype=np.float32)
    for col in range(P - 1):
        if col == d - 1:
            continue
        m[col, col] = -1.0
        m[col + 1, col] = 1.0
    return m


def _combine(partials, jpc=JPC, d=D, h=H, w=W):
    """Host-side finish: per-core [2d, acc_cols] fp32 partials -> [3]."""
    ncp = jpc // 2
    fsz = h * w
    nblk = fsz // 512
    ndrain = NDRAIN if fsz == 16384 else max(1, nblk // 4)
    col_dz = COL_C + ncp

    nslice = jpc * len(partials)
    crown = np.zeros(nslice, dtype=np.float64)
    root = np.zeros(nslice, dtype=np.float64)
    gxy_sum = 0.0
    gz_sum = 0.0
    for k, p in enumerate(partials):
        p = p.astype(np.float64)
        for c in range(ncp):
            sx = p[:, COL_SX + c]  # per-plane sum(x)
            gy = p[:, COL_GY + c]  # per-plane sum(max over h-pairs)
            gx = p[:, COL_GX + c]  # per-plane sum(max over w-pairs)
            rr = p[:, COL_R + c]  # per-plane sum(row0 + row_{h-1})
            cc = p[:, COL_C + c]  # per-plane sum(col0 + col_{w-1})
            # sum|a-b| = 2*sum(max) - sum(a) - sum(b); the signed sums
            # telescope: gy: -2*sx + rr ; gx: -2*sx + cc (per plane).
            gxy_sum += (2.0 * gy - 2.0 * sx + rr).sum()
            gxy_sum += (2.0 * gx - 2.0 * sx + cc).sum()
            for s in (0, 1):
                sl = k * jpc + 2 * c + s
                crown[sl] = sx[s * d : s * d + d // 2].sum()
                root[sl] = sx[s * d + d // 2 : s * d + d].sum()
        dz = p[:, col_dz : col_dz + ncp * ndrain]
        # rows d-1 and 2d-1 are |0| = 0 (zeroed bidiag columns)
        gz_sum += dz.sum()

    total = crown + root
    valid = (total > 0) & (root > 0)
    safe_root = np.where(root > 0, root, 1.0)
    ratio_loss = np.where(valid, (crown / safe_root - EXPECTED_RATIO) ** 2, 0.0)
    cr_loss = ratio_loss.sum() / nslice

    nxy = nslice * d * h * (w - 1)  # == nslice * d * (h-1) * w
    nz = nslice * (d - 1) * h * w
    tv = gxy_sum / nxy + gz_sum / nz

    crown_root = cr_loss * CROWN_ROOT_W
    smoothness = tv * SMOOTH_W
    return np.array(
        [crown_root, smoothness, crown_root + smoothness], dtype=np.float32
    )


def kernel(segmentation: np.ndarray) -> np.ndarray:
    global last_exec_time_ns
    from concourse.bass_utils import run_bass_kernel_spmd

    seg = np.ascontiguousarray(np.asarray(segmentation), dtype=np.float32)
    assert seg.shape == (B, C, D, H, W)
    nc = _get_program()

    bd = _bidiag_np()
    shards = seg.reshape(B * C, D, H, W)
    in_maps = [
        {"seg": np.ascontiguousarray(shards[k * JPC : (k + 1) * JPC]), "bidiag": bd}
        for k in range(NCORES)
    ]
    trace = bool(os.environ.get("BASS_TRACE"))
    res = run_bass_kernel_spmd(nc, in_maps, list(range(NCORES)), trace=trace)
    last_exec_time_ns = res.exec_time_ns
    partials = [res.results[k]["partials"] for k in range(NCORES)]
    return _combine(partials)


# revision 6
# speedup vs baseline: 1.2001x; 1.2001x over previous
"""Trainium2 Bass kernel for nn_DentalAnatomyLoss.

Computes, for segmentation [B=2, C=32, D=64, H=128, W=128] fp32:
  - crown/root ratio loss (per (b,c) sums over d<32 / d>=32)
  - 3D total-variation loss (mean |diff| along w, h, d)
  - returns stack([crown_root, smoothness, total_anatomy]) fp32 [3]

Strategy: pure data-parallel over the 64 (b,c) slices, 8 per NeuronCore.
Each core reduces its 32 MiB shard to a [128, 52] fp32 partial tensor;
the host combines partials into the 3 scalars.

Layout: d-on-partitions. Each "chunk pair" (cp) holds 2 slices:
partition p = s*64 + d for local slice s in {0,1}, plane d in 0..63;
free axis = (h, w) = 16384 bf16. Benefits over the h-partition layout:
  - DMA reads are 16 KiB contiguous per partition (vs 512 B rows), and
    the fp32->bf16 cast happens inside the SWDGE DMA (measured at full
    HBM rate), freeing ScalarE entirely from casting.
  - The h-diff (gy) becomes an aligned free-axis shift by w -> one fused
    scalar_tensor_tensor max+accum per cp on VectorE.
  - The d-diff (gz) is the partition-axis diff -> TensorE block-bidiag
    matmul into PSUM (columns 63/127 zeroed so no cross-slice pairs),
    drained by ScalarE Abs+accum. Rows 63/127 drain |0| = 0.

Per-core engine budget (measured sustained rates):
  VectorE ~136 us: gx + gy fused STT max+accum (1x; sweeping 2x modes
    does not help: any elementwise+reduce pair costs the same 2 touches).
  ScalarE ~131 us: per-plane sum(x) via broadcast-out Copy+accum (fp32
    exact, feeds crown/root and the max-trick telescopes), PSUM drains,
    and the tiny first/last row/col telescope sums.
  TensorE ~70 us, DMA ~100 us (HBM roofline ~94 us/core).

Host recovers sum|a-b| = 2*sum(max(a,b)) - sum(a) - sum(b); the signed
sums telescope to per-plane sums and first/last row/col sums. gx and gy
share one denominator (d*h*(w-1) == d*(h-1)*w), gz has its own.
"""

import os

import numpy as np

B, C, D, H, W = 2, 32, 64, 128, 128
NCORES = 8
JPC = (B * C) // NCORES  # slices per core
CROWN_ROOT_W = 2.0
SMOOTH_W = 1.5
EXPECTED_RATIO = 1.2

# accumulator column layout in the [128, ACC_COLS] partial tensor.
# V and S ops run per quarter (nq=4 h-blocks per cp) so compute streams
# right behind each quarter's DMA; the host sums quarter columns.
NCP = JPC // 2  # chunk pairs per core
NQ = 4  # quarters per chunk pair
COL_SX = 0  # NCP*NQ: per-plane sum(x), per quarter
COL_GY = COL_SX + NCP * NQ  # NCP*NQ: per-plane sum(max h-pairs), in-quarter
COL_GX = COL_GY + NCP * NQ  # NCP*NQ: per-plane sum(max w-pairs), per quarter
COL_GYB = COL_GX + NCP * NQ  # NCP*(NQ-1): boundary-row max sums
COL_R = COL_GYB + NCP * (NQ - 1)  # NCP: per-plane sum(row0 + row_{h-1})
COL_C = COL_R + NCP  # NCP*NQ: per-plane sum(col0 + col_{w-1}), per quarter
COL_DZ = COL_C + NCP * NQ  # NCP*NDRAIN: PSUM |dz| drains
NDRAIN = 8
ACC_COLS = COL_DZ + NCP * NDRAIN

_PROG_CACHE: dict = {}
last_exec_time_ns = None


def _build_program(jpc=JPC, d=D, h=H, w=W, repeat=1, skip=()):
    """Build the (single) SPMD Bass program run identically on all cores.

    repeat>1 wraps the whole compute in a hardware For_i loop (identical
    result, used only for wall-clock timing of the kernel body).
    """
    from contextlib import ExitStack

    import concourse.tile as tile
    from concourse import bacc, mybir

    f32 = mybir.dt.float32
    bf16 = mybir.dt.bfloat16
    AO = mybir.AluOpType
    AF = mybir.ActivationFunctionType

    ncp = jpc // 2
    P = 2 * d  # partitions per chunk pair
    fsz = h * w  # free size per partition (one (h,w) plane)
    nq = 4  # DMA splits per chunk pair
    qsz = fsz // nq
    nblk = fsz // 512  # 512-col matmul blocks per cp
    ndrain = NDRAIN if fsz == 16384 else nq
    blk_per_drain = nblk // ndrain

    col_gyb = COL_GX + ncp * nq
    col_r = col_gyb + ncp * (nq - 1)
    col_c = col_r + ncp
    col_dz = col_c + ncp * nq
    acc_cols = col_dz + ncp * ndrain

    nc = bacc.Bacc(
        "TRN2",
        target_bir_lowering=False,
        debug=False,
        enable_asserts=False,
        num_devices=NCORES,
    )
    seg = nc.dram_tensor("seg", [jpc, d, h, w], f32, kind="ExternalInput").ap()
    bd = nc.dram_tensor("bidiag", [P, P], bf16, kind="ExternalInput").ap()
    out = nc.dram_tensor("partials", [P, acc_cols], f32, kind="ExternalOutput").ap()

    with tile.TileContext(nc) as tc, ExitStack() as ctx:
        singles = ctx.enter_context(tc.tile_pool(name="singles", bufs=1))
        xbp = ctx.enter_context(tc.tile_pool(name="xb", bufs=3))
        scrp = ctx.enter_context(tc.tile_pool(name="scr", bufs=2))
        dumbp = ctx.enter_context(tc.tile_pool(name="dumb", bufs=2))
        psp = ctx.enter_context(tc.tile_pool(name="ps", bufs=2, space="PSUM"))

        bd_sb = singles.tile([P, P], bf16)
        nc.sync.dma_start(out=bd_sb, in_=bd)
        acc = singles.tile([P, acc_cols], f32)
        nc.vector.memset(acc, 0.0)

        def cp_body(c):
            # 1) SWDGE cast-DMA loads: fp32 HBM -> bf16 SBUF, d-layout.
            #    Per partition: contiguous 4*qsz bytes from DRAM.
            xb = xbp.tile([P, fsz], bf16)
            src = seg[2 * c : 2 * c + 2].rearrange("s d h w -> (s d) (h w)")
            for q in range(nq):
                nc.gpsimd.dma_start(
                    out=xb[:, q * qsz : (q + 1) * qsz],
                    in_=src[:, q * qsz : (q + 1) * qsz],
                )

            scratch = scrp.tile([P, qsz], bf16)
            dummy = dumbp.tile([P, 1], bf16)
            hq = qsz // w  # h-rows per quarter
            xb3 = xb.rearrange("p (r c2) -> p r c2", c2=w)
            scr3 = scratch.rearrange("p (r c2) -> p r c2", c2=w)

            # 2-4) Per-quarter compute, streamed behind each quarter's DMA:
            #   VectorE: fused max+accum for gy (shift by w, in-quarter) and
            #     gx (shift by 1 inside each w-row). Both 1x; one op each.
            #   ScalarE: per-plane sum(x) + col0/col_{w-1} telescope sums.
            #   TensorE block-bidiag d-diffs -> PSUM; ScalarE Abs drains.
            # Interior first/last-row telescope sums cancel against the
            # quarter-boundary terms, so only the cp-level row0+row_{h-1}
            # op and 3 tiny boundary-row max ops are needed.
            for q in range(nq):
                qc = nq * c + q
                r0 = q * hq
                if "gy" not in skip:
                    nc.vector.scalar_tensor_tensor(
                        out=scratch[:, 0 : qsz - w],
                        in0=xb[:, q * qsz + w : (q + 1) * qsz],
                        scalar=0.0,
                        in1=xb[:, q * qsz : (q + 1) * qsz - w],
                        op0=AO.bypass,
                        op1=AO.max,
                        accum_out=acc[:, COL_GY + qc : COL_GY + qc + 1],
                    )
                    if q > 0:
                        # boundary pair: last row of q-1, first row of q
                        nc.vector.scalar_tensor_tensor(
                            out=scr3[:, 0, :],
                            in0=xb3[:, r0, :],
                            scalar=0.0,
                            in1=xb3[:, r0 - 1, :],
                            op0=AO.bypass,
                            op1=AO.max,
                            accum_out=acc[
                                :,
                                col_gyb + (nq - 1) * c + q - 1 : col_gyb
                                + (nq - 1) * c
                                + q,
                            ],
                        )
                if "gx" not in skip:
                    nc.vector.scalar_tensor_tensor(
                        out=scr3[:, 0:hq, 0 : w - 1],
                        in0=xb3[:, r0 : r0 + hq, 1:w],
                        scalar=0.0,
                        in1=xb3[:, r0 : r0 + hq, 0 : w - 1],
                        op0=AO.bypass,
                        op1=AO.max,
                        accum_out=acc[:, COL_GX + qc : COL_GX + qc + 1],
                    )
                if "sx" not in skip:
                    nc.scalar.activation(
                        out=dummy.broadcast_to((P, qsz)),
                        in_=xb[:, q * qsz : (q + 1) * qsz],
                        func=AF.Copy,
                        accum_out=acc[:, COL_SX + qc : COL_SX + qc + 1],
                    )
                    cols = xb.rearrange("p (r c2) -> p c2 r", c2=w)[
                        :, 0 : w : w - 1, r0 : r0 + hq
                    ]
                    nc.scalar.activation(
                        out=dummy.broadcast_to((P, 2, hq)),
                        in_=cols,
                        func=AF.Copy,
                        accum_out=acc[:, col_c + qc : col_c + qc + 1],
                    )
                if "gz" not in skip:
                    dr_per_q = ndrain // nq
                    for t in range(dr_per_q):
                        ps = psp.tile([P, blk_per_drain, 512], f32)
                        for b in range(blk_per_drain):
                            blk = (q * dr_per_q + t) * blk_per_drain + b
                            nc.tensor.matmul(
                                ps[:, b, :],
                                bd_sb,
                                xb[:, blk * 512 : (blk + 1) * 512],
                                start=True,
                                stop=True,
                            )
                        col = col_dz + ndrain * c + q * dr_per_q + t
                        nc.scalar.activation(
                            out=dummy.broadcast_to((P, blk_per_drain, 512)),
                            in_=ps[:, :, :],
                            func=AF.Abs,
                            accum_out=acc[:, col : col + 1],
                        )
            if "sx" not in skip:
                # sum(row0 + row_{h-1}) per plane (cp-level; interior
                # quarter rows telescoped away)
                rows = xb3[:, 0 : h : h - 1, :]
                nc.scalar.activation(
                    out=dummy.broadcast_to((P, 2, w)),
                    in_=rows,
                    func=AF.Copy,
                    accum_out=acc[:, col_r + c : col_r + c + 1],
                )

        def all_cps():
            for c in range(ncp):
                cp_body(c)

        if repeat == 1:
            all_cps()
        else:
            with tc.For_i(0, repeat, 1):
                all_cps()
        nc.sync.dma_start(out=out, in_=acc)

    nc.compile()
    return nc


def _get_program():
    key = "full"
    if key not in _PROG_CACHE:
        _PROG_CACHE[key] = _build_program()
    return _PROG_CACHE[key]


def _bidiag_np(d=D):
    """lhsT for the d-diff matmul: out[m,:] = x[m+1,:] - x[m,:] within
    each slice; columns d-1 and 2d-1 zeroed (no cross-slice pairs)."""
    import ml_dtypes

    P = 2 * d
    m = np.zeros((P, P), dtype=np.float32)
    for col in range(P - 1):
        if col == d - 1:
            continue
        m[col, col] = -1.0
        m[col + 1, col] = 1.0
    return m.astype(ml_dtypes.bfloat16)


def _combine(partials, jpc=JPC, d=D, h=H, w=W):
    """Host-side finish: per-core [2d, acc_cols] fp32 partials -> [3]."""
    ncp = jpc // 2
    fsz = h * w
    nq = NQ
    nblk = fsz // 512
    ndrain = NDRAIN if fsz == 16384 else nq
    col_gyb = COL_GX + ncp * nq
    col_r = col_gyb + ncp * (nq - 1)
    col_c = col_r + ncp
    col_dz = col_c + ncp * nq

    nslice = jpc * len(partials)
    crown = np.zeros(nslice, dtype=np.float64)
    root = np.zeros(nslice, dtype=np.float64)
    gxy_sum = 0.0
    gz_sum = 0.0
    for k, p in enumerate(partials):
        p = p.astype(np.float64)
        for c in range(ncp):
            qs = slice(nq * c, nq * c + nq)
            sx = p[:, COL_SX + nq * c : COL_SX + nq * c + nq].sum(axis=1)
            gy = p[:, COL_GY + nq * c : COL_GY + nq * c + nq].sum(axis=1)
            gyb = p[
                :, col_gyb + (nq - 1) * c : col_gyb + (nq - 1) * c + nq - 1
            ].sum(axis=1)
            gx = p[:, COL_GX + nq * c : COL_GX + nq * c + nq].sum(axis=1)
            rr = p[:, col_r + c]  # per-plane sum(row0 + row_{h-1})
            cc = p[:, col_c + nq * c : col_c + nq * c + nq].sum(axis=1)
            # sum|a-b| = 2*sum(max) - sum(a) - sum(b); the signed sums
            # telescope: gy: -2*sx + rr ; gx: -2*sx + cc (per plane).
            # In-quarter gy maxes + boundary-row maxes cover all h-pairs.
            gxy_sum += (2.0 * (gy + gyb) - 2.0 * sx + rr).sum()
            gxy_sum += (2.0 * gx - 2.0 * sx + cc).sum()
            for s in (0, 1):
                sl = k * jpc + 2 * c + s
                crown[sl] = sx[s * d : s * d + d // 2].sum()
                root[sl] = sx[s * d + d // 2 : s * d + d].sum()
        dz = p[:, col_dz : col_dz + ncp * ndrain]
        # rows d-1 and 2d-1 are |0| = 0 (zeroed bidiag columns)
        gz_sum += dz.sum()

    total = crown + root
    valid = (total > 0) & (root > 0)
    safe_root = np.where(root > 0, root, 1.0)
    ratio_loss = np.where(valid, (crown / safe_root - EXPECTED_RATIO) ** 2, 0.0)
    cr_loss = ratio_loss.sum() / nslice

    nxy = nslice * d * h * (w - 1)  # == nslice * d * (h-1) * w
    nz = nslice * (d - 1) * h * w
    tv = gxy_sum / nxy + gz_sum / nz

    crown_root = cr_loss * CROWN_ROOT_W
    smoothness = tv * SMOOTH_W
    return np.array(
        [crown_root, smoothness, crown_root + smoothness], dtype=np.float32
    )


def kernel(segmentation: np.ndarray) -> np.ndarray:
    global last_exec_time_ns
    from concourse.bass_utils import run_bass_kernel_spmd

    seg = np.ascontiguousarray(np.asarray(segmentation), dtype=np.float32)
    assert seg.shape == (B, C, D, H, W)
    nc = _get_program()

    bd = _bidiag_np()
    shards = seg.reshape(B * C, D, H, W)
    in_maps = [
        {"seg": np.ascontiguousarray(shards[k * JPC : (k + 1) * JPC]), "bidiag": bd}
        for k in range(NCORES)
    ]
    trace = bool(os.environ.get("BASS_TRACE"))
    res = run_bass_kernel_spmd(nc, in_maps, list(range(NCORES)), trace=trace)
    last_exec_time_ns = res.exec_time_ns
    partials = [res.results[k]["partials"] for k in range(NCORES)]
    return _combine(partials)
